# revision 3
# baseline (speedup 1.0000x reference)
"""MinusAttention kernel for Trainium2 (8 NeuronCores, Bass/Tile) — v2.

Math: score[i,j] = (w.q_i - w.k_j + b) / sqrt(E) with causal mask.
Within a softmax row i the w.q_i and b terms cancel, so

    weights[i,j] = g_j / sum_{j'<=i} g_j',   g_j = exp(-w.k_j / sqrt(E))
    out[i,:]     = (sum_{j<=i} g_j V[j,:]) / (sum_{j<=i} g_j)

i.e. a causal cumulative weighted average of V -- O(S*E) per (b,h) --
and the output does not depend on queries at all.

Structure vs the 38.0us staged baseline (now ~35.1us median, best 34.4):
  - output stores on the HW DGE rings (scalar Q10; last pair split
    across both rings) instead of the ~80GB/s gpsimd software queue;
    the sync ring is reserved for the scatters (a 256KB store ahead of
    a scatter in the Q1 FIFO stalls the rm chain)
  - tail software-pipelined in two stages so a pair's c32 block-sum
    drain (the scatter and rm chain hang off it) never queues behind
    the previous pair's 1.1us cw copy in the ACT stream; order edges
    keep the next pair's rm ahead of the current normalize on DVE
  - the last pair normalizes straight from PSUM per carry-chunk (no
    ACT cw hop on the drain chain) and stores each half as it is ready
  - one exp writes the pairwise-duplicated g4 directly (stride-0 bcast
    src on ACT); the psd matmul reads a strided g4 view
  - consts packed/placed so nothing stalls: tri+ones lead the scalar
    ring, mke rides the scalar ring behind vg1; kt all on the sync
    ring (a kt trigger on the scalar ring stalls exp0 behind a reused
    DMA semaphore), kt0 split in halves for an earlier first reduce
"""

import numpy as np

B, L, S, H, E = 4, 2048, 2048, 8, 64
NCORES = 8
PAIRS = (B * H) // NCORES  # 4 (b,h) pairs per core
NBLK = S // 128  # 16
CHUNKS = [(0, 8), (8, 16)]
SCALE = np.float32(1.0 / np.sqrt(np.float32(E)))

TRACE = False
LAST_RESULTS = None

_compiled = None


def _build():
    from concourse import bacc
    import concourse.mybir as mybir
    import concourse.tile as tile

    f16 = mybir.dt.float16
    f32 = mybir.dt.float32
    nc = bacc.Bacc("TRN2", target_bir_lowering=False, debug=False)

    ktw = nc.dram_tensor("ktw", [PAIRS, 128, NBLK, E], f16, kind="ExternalInput")
    vg = nc.dram_tensor("vg", [PAIRS, 128, NBLK, E], f16, kind="ExternalInput")
    # packed: cols 0:128 = tri (tri[c,p]=1 iff c<=p); cols 128:256 rows
    # 0:16 = ones16 (lhsT for the inter-block carry)
    to_c = nc.dram_tensor("to_c", [128, 256], f16, kind="ExternalInput")
    # maskKED[k', k, e] = 1 iff k' < k (bcast along e incl. the D col)
    mke_c = nc.dram_tensor("mke_c", [16, NBLK, E + 1], f16, kind="ExternalInput")
    out = nc.dram_tensor("out", [PAIRS, 128, NBLK, E], f16, kind="ExternalOutput")

    with tile.TileContext(nc) as tc:
        with (
            nc.allow_low_precision(reason="fp16 kernel; harness gate is 2e-2"),
            tc.tile_pool(name="const", bufs=1) as cpool,
            tc.tile_pool(name="ktp", bufs=PAIRS) as ktp,
            tc.tile_pool(name="vgp", bufs=PAIRS) as vgp,
            tc.tile_pool(name="skp", bufs=4) as skp,
            tc.tile_pool(name="gp", bufs=4) as gp,
            tc.tile_pool(name="wgp", bufs=PAIRS) as wgp,
            tc.tile_pool(name="bsp", bufs=4) as bsp,
            tc.tile_pool(name="bstp", bufs=4) as bstp,
            tc.tile_pool(name="rmp", bufs=4) as rmp,
            tc.tile_pool(name="rp", bufs=4) as rp,
            tc.tile_pool(name="cwp", bufs=4) as cwp,
            tc.tile_pool(name="otp", bufs=4) as otp,
            tc.tile_pool(name="ps", bufs=3, space="PSUM") as psp,
            tc.tile_pool(name="psd", bufs=2, space="PSUM") as psdp,
        ):
            allp = list(range(PAIRS))
            kts, vgts = {}, {}
            tos = cpool.tile([128, 256], f16)
            maskKED = cpool.tile([16, NBLK, E + 1], f16)
            tri = tos[:, 0:128]
            ones16 = tos[0:16, 128:256]
            # Sync ring: kt0 (split), kt1, consts, kt2; Scalar ring: vg0-3,
            # kt3.  Trigger instructions cost ~0.6us of engine time each,
            # so front triggers are split across both engines.
            nc.scalar.dma_start(out=tos[:], in_=to_c[:])
            for p in allp:
                kt = ktp.tile([128, NBLK, E // 4, 4], f16, tag="kt")
                vgt = vgp.tile([128, NBLK, E // 4, 4], f16, tag="vg")
                ktv = kt[:].rearrange("p k a b -> p k (a b)")
                if p == 0:
                    nc.sync.dma_start(out=ktv[:, 0:8, :], in_=ktw[p][:, 0:8, :])
                    nc.sync.dma_start(out=ktv[:, 8:16, :], in_=ktw[p][:, 8:16, :])
                else:
                    nc.sync.dma_start(out=ktv, in_=ktw[p])
                nc.scalar.dma_start(
                    out=vgt[:].rearrange("p k a b -> p k (a b)"), in_=vg[p])
                kts[p], vgts[p] = kt, vgt
                if p == 1:
                    nc.scalar.dma_start(out=maskKED[:], in_=mke_c[:])

            # front phases, paced pair-by-pair on DVE/ACT; the order edge
            # keeps the scheduler from running later pairs' reduces (gated
            # on late kt DMAs) ahead of this pair's wg in the DVE stream
            from concourse.tile_rust import add_dep_helper
            wgs, g4s = {}, {}
            prev_wg = None
            for p in allp:
                t1 = skp.tile([128, NBLK, E // 8, 4], f16, tag="t1")
                sk = skp.tile([128, NBLK], f16, tag="sk")
                red = None
                for k0, k1 in (CHUNKS if p == 0 else [(0, NBLK)]):
                    red = nc.vector.tensor_tensor(
                        out=t1[:, k0:k1], in0=kts[p][:, k0:k1, 0:E // 8, :],
                        in1=kts[p][:, k0:k1, E // 8:E // 4, :],
                        op=mybir.AluOpType.add,
                    )
                    nc.vector.tensor_reduce(
                        sk[:, k0:k1], t1[:, k0:k1], mybir.AxisListType.XY,
                        mybir.AluOpType.add,
                    )
                if prev_wg is not None:
                    add_dep_helper(red.ins, prev_wg.ins, sync=False,
                                   reason="reduce after prev pair wg")
                # one exp writes the pairwise-duplicated g4 directly; its
                # (stride-1, count-4) last dim keeps the big broadcast
                # multiplies in DVE packed-16 mode
                g4 = gp.tile([128, NBLK, 1, 4], f16, tag="g4")
                nc.scalar.activation(
                    g4[:].rearrange("p k a b -> p k (a b)"),
                    sk[:].to_broadcast([128, NBLK, 4]),
                    mybir.ActivationFunctionType.Exp,
                )
                wg = wgp.tile([128, NBLK, E // 4, 4], f16, tag="wg")
                prev_wg = nc.vector.tensor_tensor(
                    out=wg[:], in0=vgts[p][:],
                    in1=g4[:].broadcast_to([128, NBLK, E // 4, 4]),
                    op=mybir.AluOpType.mult,
                )
                wgs[p] = wg
                g4s[p] = g4

            # per-pair tail, software-pipelined in two stages so the ACT
            # stream interleaves by ready-time (a pair's c32 drain must not
            # queue behind the previous pair's 1.1us cw copy -- the scatter
            # and rm chain hang off it)
            def stage1(p):
                # D prefix first: needs only g4, runs while wg is built
                psd = psdp.tile([128, NBLK], f32, tag="psd")
                nc.tensor.matmul(
                    psd[:], lhsT=tri,
                    rhs=g4s[p][:, :, 0, 0:1].rearrange("p k o -> p (k o)"),
                    start=True, stop=False, skip_group_check=True,
                )
                ps = psp.tile([128, NBLK, E], f32, tag="ps")
                for k0, k1 in CHUNKS:
                    nc.tensor.matmul(
                        ps[:, k0:k1, :], lhsT=tri,
                        rhs=wgs[p][:, k0:k1, :, :].rearrange(
                            "p k a b -> p k (a b)"),
                        start=True, stop=False, skip_group_check=True,
                    )

                # block sums live in ps row 127; PSUM reads need a
                # 32-aligned partition base: copy rows 96:128 (ACT), then
                # scatter row 31 to partitions
                c32 = bsp.tile([32, NBLK, E + 1], f16, tag="c32")
                nc.scalar.copy(c32[:, :, 0:E], ps[96:128, :, :])
                nc.scalar.copy(
                    c32[:, :, E:E + 1].rearrange("p k o -> p (k o)"),
                    psd[96:128, :])
                bsT = bstp.tile([NBLK, 1, E + 1], f16, tag="bs")
                nc.sync.dma_start(out=bsT[:], in_=c32[31:32, :, :],
                                  single_packet=True)

                # rmD first: the D chain (carry -> recip -> normalize) is
                # the longer pole
                rmD = rmp.tile([NBLK, NBLK], f16, tag="rmd")
                nc.vector.tensor_tensor(
                    out=rmD[:],
                    in0=maskKED[:, :, E:E + 1].rearrange("a k o -> a (k o)"),
                    in1=bsT[:, :, E:E + 1].rearrange(
                        "a k o -> a (k o)").broadcast_to([NBLK, NBLK]),
                    op=mybir.AluOpType.mult,
                )
                rmV = rmp.tile([NBLK, NBLK, E], f16, tag="rmv")
                rmv_ins = nc.vector.tensor_tensor(
                    out=rmV[:], in0=maskKED[:, :, 0:E],
                    in1=bsT[:, :, 0:E].broadcast_to([NBLK, NBLK, E]),
                    op=mybir.AluOpType.mult,
                )

                # inter-block carries on PE
                nc.tensor.matmul(
                    psd[:], lhsT=ones16, rhs=rmD[:],
                    start=False, stop=True, skip_group_check=True,
                )
                for k0, k1 in CHUNKS:
                    nc.tensor.matmul(
                        ps[:, k0:k1, :], lhsT=ones16,
                        rhs=rmV[:, k0:k1, :],
                        start=False, stop=True, skip_group_check=True,
                    )

                # reciprocal on DVE
                r = rp.tile([128, NBLK], f16, tag="r")
                nc.vector.reciprocal(r[:], psd[:])
                return ps, r, rmv_ins

            def stage2(p, ps, r, next_rmv=None):
                # r4 duplication on DVE (an ACT hop here can queue
                # behind big c32/cw copies and stall the normalize)
                r4 = rp.tile([128, NBLK, 1, 4], f16, tag="r4")
                nc.vector.tensor_copy(
                    r4[:].rearrange("p k a b -> p k (a b)"),
                    r[:].to_broadcast([128, NBLK, 4]))
                # normalize straight from PSUM per carry-chunk (no ACT cw
                # hop -- the DVE->PE->ACT->DVE ping-pong left DVE idle);
                # output stores on the scalar HW ring (the sync ring
                # carries the scatters), last pair split across both rings
                ot = otp.tile([128, NBLK, E // 4, 4], f16, tag="ot")
                otv = ot[:].rearrange("p k a b -> p k (a b)")
                if p >= PAIRS - 1:
                    # the last pair gates the drain: normalize straight
                    # from PSUM per carry-chunk (no ACT cw hop)
                    for ci, (k0, k1) in enumerate(CHUNKS):
                        norm = nc.vector.tensor_tensor(
                            out=ot[:, k0:k1],
                            in0=ps[:, k0:k1, :].rearrange(
                                "p k (a b) -> p k a b", b=4),
                            in1=r4[:, k0:k1].broadcast_to(
                                [128, 8, E // 4, 4]),
                            op=mybir.AluOpType.mult,
                        )
                        if ci == 0 and next_rmv is not None:
                            add_dep_helper(norm.ins, next_rmv.ins,
                                           sync=False,
                                           reason="norm after next rm")
                        if p == PAIRS - 1:
                            if ci == 0:
                                nc.scalar.dma_start(
                                    out=out[p][:, k0:k1, :],
                                    in_=otv[:, k0:k1, :])
                            else:
                                nc.sync.dma_start(
                                    out=out[p][:, k0:k1, :],
                                    in_=otv[:, k0:k1, :])
                    if p < PAIRS - 1:
                        nc.scalar.dma_start(out=out[p], in_=otv)
                else:
                    # early pairs: ACT cw drain overlaps under later pairs
                    cw = cwp.tile([128, NBLK, E // 4, 4], f16, tag="cw")
                    nc.scalar.copy(
                        cw[:].rearrange("p k a b -> p k (a b)"), ps[:])
                    norm = nc.vector.tensor_tensor(
                        out=ot[:], in0=cw[:],
                        in1=r4[:].broadcast_to([128, NBLK, E // 4, 4]),
                        op=mybir.AluOpType.mult,
                    )
                    if next_rmv is not None:
                        add_dep_helper(norm.ins, next_rmv.ins, sync=False,
                                       reason="norm after next rm")
                    nc.scalar.dma_start(out=out[p], in_=otv)

            pending = None
            for p in allp:
                s1 = stage1(p)
                if pending is not None:
                    stage2(pending[0], pending[1][0], pending[1][1],
                           next_rmv=s1[2])
                pending = (p, s1)
            stage2(pending[0], pending[1][0], pending[1][1])

    nc.compile()
    return nc


def _get_compiled():
    global _compiled
    if _compiled is None:
        _compiled = _build()
    return _compiled


def _consts():
    f16 = np.float16
    to = np.zeros((128, 256), np.float32)
    to[:, 0:128] = np.triu(np.ones((128, 128), np.float32))  # tri[c,p]=1 iff c<=p
    to[0:16, 128:256] = 1.0  # ones16 as lhsT rows 0:16
    mk = (np.arange(NBLK)[:, None] < np.arange(NBLK)[None, :]).astype(np.float32)
    mke = np.broadcast_to(mk[:, :, None], (16, NBLK, E + 1)).astype(f16)
    return {
        "to_c": to.astype(f16),
        "mke_c": np.ascontiguousarray(mke),
    }


def prep_inputs(keys: np.ndarray, values: np.ndarray, w_score: np.ndarray):
    """Host-side reshard: returns in_maps (list of 8 dicts)."""
    keys = np.asarray(keys, dtype=np.float32)
    values = np.asarray(values, dtype=np.float32)
    w = np.asarray(w_score, dtype=np.float32)

    # [B,S,H,E] -> [B,H,S,E] -> [B*H, NBLK, 128, E] -> [B*H, 128, NBLK, E]
    kt = keys.transpose(0, 2, 1, 3).reshape(B * H, NBLK, 128, E)
    kt = (kt * (-SCALE * w)).transpose(0, 2, 1, 3).astype(np.float16)

    vgf = values.transpose(0, 2, 1, 3).reshape(B * H, NBLK, 128, E)
    vgf = vgf.transpose(0, 2, 1, 3).astype(np.float16)  # [B*H, 128, NBLK, E]

    consts = _consts()
    in_maps = []
    for c in range(NCORES):
        sl = slice(PAIRS * c, PAIRS * (c + 1))
        m = {
            "ktw": np.ascontiguousarray(kt[sl]),
            "vg": np.ascontiguousarray(vgf[sl]),
        }
        m.update(consts)
        in_maps.append(m)
    return in_maps


def assemble_output(results) -> np.ndarray:
    # results[c]["out"]: [PAIRS, 128, NBLK, E]; s = 128*k + partition
    arr = np.stack([np.asarray(r["out"]) for r in results])
    arr = arr.reshape(B * H, 128, NBLK, E).astype(np.float32)
    arr = arr.transpose(0, 2, 1, 3).reshape(B, H, L, E).transpose(0, 2, 1, 3)
    return np.ascontiguousarray(arr)


def kernel(queries=None, keys=None, values=None, w_score=None, b_score=None, attn_mask=None, **_):
    global LAST_RESULTS
    from concourse.bass_utils import run_bass_kernel_spmd

    nc = _get_compiled()
    in_maps = prep_inputs(keys, values, w_score)
    res = run_bass_kernel_spmd(nc, in_maps, core_ids=list(range(NCORES)), trace=TRACE)
    LAST_RESULTS = res
    return assemble_output(res.results)


# revision 4
# speedup vs baseline: 1.0721x; 1.0721x over previous
"""MinusAttention kernel for Trainium2 (8 NeuronCores, Bass/Tile) — v2.

Math: score[i,j] = (w.q_i - w.k_j + b) / sqrt(E) with causal mask.
Within a softmax row i the w.q_i and b terms cancel, so

    weights[i,j] = g_j / sum_{j'<=i} g_j',   g_j = exp(-w.k_j / sqrt(E))
    out[i,:]     = (sum_{j<=i} g_j V[j,:]) / (sum_{j<=i} g_j)

i.e. a causal cumulative weighted average of V -- O(S*E) per (b,h) --
and the output does not depend on queries at all.

Structure vs the 38.0us staged baseline (now ~35.5us median, best 34.4):
  - output stores on the HW DGE rings (scalar Q10; last pair split
    across both rings) instead of the ~80GB/s gpsimd software queue;
    the sync ring is reserved for the scatters (a 256KB store ahead of
    a scatter in the Q1 FIFO stalls the rm chain)
  - tail software-pipelined in two stages so a pair's c32 block-sum
    drain (the scatter and rm chain hang off it) never queues behind
    the previous pair's 1.1us cw copy in the ACT stream; order edges
    keep the next pair's rm ahead of the current normalize on DVE
  - the last pair normalizes straight from PSUM per carry-chunk (no
    ACT cw hop on the drain chain) and stores each half as it is ready
  - one fused mask-multiply builds the V and D carry operands at once
    (66-wide, even last dim keeps DVE packed-16 mode; strided carry
    matmul rhs views are free)
  - one exp writes the pairwise-duplicated g4 directly (stride-0 bcast
    src on ACT); the psd matmul reads a strided g4 view
  - consts packed/placed so nothing stalls: tri+ones lead the scalar
    ring, mke rides the scalar ring behind vg1; kt all on the sync
    ring (a kt trigger on the scalar ring stalls exp0 behind a reused
    DMA semaphore), kt0 split in halves for an earlier first reduce
"""

import numpy as np

B, L, S, H, E = 4, 2048, 2048, 8, 64
NCORES = 8
PAIRS = (B * H) // NCORES  # 4 (b,h) pairs per core
NBLK = S // 128  # 16
CHUNKS = [(0, 8), (8, 16)]
SCALE = np.float32(1.0 / np.sqrt(np.float32(E)))

TRACE = False
LAST_RESULTS = None

_compiled = None


def _build():
    from concourse import bacc
    import concourse.mybir as mybir
    import concourse.tile as tile

    f16 = mybir.dt.float16
    f32 = mybir.dt.float32
    nc = bacc.Bacc("TRN2", target_bir_lowering=False, debug=False)

    ktw = nc.dram_tensor("ktw", [PAIRS, 128, NBLK, E], f16, kind="ExternalInput")
    vg = nc.dram_tensor("vg", [PAIRS, 128, NBLK, E], f16, kind="ExternalInput")
    # packed: cols 0:128 = tri (tri[c,p]=1 iff c<=p); cols 128:256 rows
    # 0:16 = ones16 (lhsT for the inter-block carry)
    to_c = nc.dram_tensor("to_c", [128, 256], f16, kind="ExternalInput")
    # maskKED[k', k, e] = 1 iff k' < k (bcast along e incl. the D col)
    mke_c = nc.dram_tensor("mke_c", [16, NBLK, E + 2], f16, kind="ExternalInput")
    out = nc.dram_tensor("out", [PAIRS, 128, NBLK, E], f16, kind="ExternalOutput")

    with tile.TileContext(nc) as tc:
        with (
            nc.allow_low_precision(reason="fp16 kernel; harness gate is 2e-2"),
            tc.tile_pool(name="const", bufs=1) as cpool,
            tc.tile_pool(name="ktp", bufs=PAIRS) as ktp,
            tc.tile_pool(name="vgp", bufs=PAIRS) as vgp,
            tc.tile_pool(name="skp", bufs=4) as skp,
            tc.tile_pool(name="gp", bufs=4) as gp,
            tc.tile_pool(name="wgp", bufs=PAIRS) as wgp,
            tc.tile_pool(name="bsp", bufs=4) as bsp,
            tc.tile_pool(name="bstp", bufs=4) as bstp,
            tc.tile_pool(name="rmp", bufs=4) as rmp,
            tc.tile_pool(name="rp", bufs=4) as rp,
            tc.tile_pool(name="cwp", bufs=4) as cwp,
            tc.tile_pool(name="otp", bufs=4) as otp,
            tc.tile_pool(name="ps", bufs=3, space="PSUM") as psp,
            tc.tile_pool(name="psd", bufs=2, space="PSUM") as psdp,
        ):
            allp = list(range(PAIRS))
            kts, vgts = {}, {}
            tos = cpool.tile([128, 256], f16)
            maskKED = cpool.tile([16, NBLK, E + 2], f16)
            tri = tos[:, 0:128]
            ones16 = tos[0:16, 128:256]
            # Sync ring: kt0 (split), kt1, consts, kt2; Scalar ring: vg0-3,
            # kt3.  Trigger instructions cost ~0.6us of engine time each,
            # so front triggers are split across both engines.
            nc.scalar.dma_start(out=tos[:], in_=to_c[:])
            for p in allp:
                kt = ktp.tile([128, NBLK, E // 4, 4], f16, tag="kt")
                vgt = vgp.tile([128, NBLK, E // 4, 4], f16, tag="vg")
                ktv = kt[:].rearrange("p k a b -> p k (a b)")
                if p == 0:
                    nc.sync.dma_start(out=ktv[:, 0:8, :], in_=ktw[p][:, 0:8, :])
                    nc.sync.dma_start(out=ktv[:, 8:16, :], in_=ktw[p][:, 8:16, :])
                else:
                    nc.sync.dma_start(out=ktv, in_=ktw[p])
                nc.scalar.dma_start(
                    out=vgt[:].rearrange("p k a b -> p k (a b)"), in_=vg[p])
                kts[p], vgts[p] = kt, vgt
                if p == 1:
                    nc.scalar.dma_start(out=maskKED[:], in_=mke_c[:])

            # front phases, paced pair-by-pair on DVE/ACT; the order edge
            # keeps the scheduler from running later pairs' reduces (gated
            # on late kt DMAs) ahead of this pair's wg in the DVE stream
            from concourse.tile_rust import add_dep_helper
            wgs, g4s = {}, {}
            prev_wg = None
            for p in allp:
                t1 = skp.tile([128, NBLK, E // 8, 4], f16, tag="t1")
                sk = skp.tile([128, NBLK], f16, tag="sk")
                red = None
                for k0, k1 in (CHUNKS if p == 0 else [(0, NBLK)]):
                    red = nc.vector.tensor_tensor(
                        out=t1[:, k0:k1], in0=kts[p][:, k0:k1, 0:E // 8, :],
                        in1=kts[p][:, k0:k1, E // 8:E // 4, :],
                        op=mybir.AluOpType.add,
                    )
                    nc.vector.tensor_reduce(
                        sk[:, k0:k1], t1[:, k0:k1], mybir.AxisListType.XY,
                        mybir.AluOpType.add,
                    )
                if prev_wg is not None:
                    add_dep_helper(red.ins, prev_wg.ins, sync=False,
                                   reason="reduce after prev pair wg")
                # one exp writes the pairwise-duplicated g4 directly; its
                # (stride-1, count-4) last dim keeps the big broadcast
                # multiplies in DVE packed-16 mode
                g4 = gp.tile([128, NBLK, 1, 4], f16, tag="g4")
                nc.scalar.activation(
                    g4[:].rearrange("p k a b -> p k (a b)"),
                    sk[:].to_broadcast([128, NBLK, 4]),
                    mybir.ActivationFunctionType.Exp,
                )
                wg = wgp.tile([128, NBLK, E // 4, 4], f16, tag="wg")
                prev_wg = nc.vector.tensor_tensor(
                    out=wg[:], in0=vgts[p][:],
                    in1=g4[:].broadcast_to([128, NBLK, E // 4, 4]),
                    op=mybir.AluOpType.mult,
                )
                wgs[p] = wg
                g4s[p] = g4

            # per-pair tail, software-pipelined in two stages so the ACT
            # stream interleaves by ready-time (a pair's c32 drain must not
            # queue behind the previous pair's 1.1us cw copy -- the scatter
            # and rm chain hang off it)
            def stage1(p):
                # D prefix first: needs only g4, runs while wg is built
                psd = psdp.tile([128, NBLK], f32, tag="psd")
                nc.tensor.matmul(
                    psd[:], lhsT=tri,
                    rhs=g4s[p][:, :, 0, 0:1].rearrange("p k o -> p (k o)"),
                    start=True, stop=False, skip_group_check=True,
                )
                ps = psp.tile([128, NBLK, E], f32, tag="ps")
                for k0, k1 in CHUNKS:
                    nc.tensor.matmul(
                        ps[:, k0:k1, :], lhsT=tri,
                        rhs=wgs[p][:, k0:k1, :, :].rearrange(
                            "p k a b -> p k (a b)"),
                        start=True, stop=False, skip_group_check=True,
                    )

                # block sums live in ps row 127; PSUM reads need a
                # 32-aligned partition base: copy rows 96:128 (ACT), then
                # scatter row 31 to partitions
                c32 = bsp.tile([32, NBLK, E + 1], f16, tag="c32")
                nc.scalar.copy(c32[:, :, 0:E], ps[96:128, :, :])
                nc.scalar.copy(
                    c32[:, :, E:E + 1].rearrange("p k o -> p (k o)"),
                    psd[96:128, :])
                bsT = bstp.tile([NBLK, 1, E + 2], f16, tag="bs")
                nc.sync.dma_start(out=bsT[:, :, 0:E + 1], in_=c32[31:32, :, :],
                                  single_packet=True)

                # one fused mask-multiply covers V (cols 0:E), the D
                # col (E), and a junk col (E+1, never read) that keeps the
                # last dim even for the DVE packed-16 mode
                rmV = rmp.tile([NBLK, NBLK, E + 2], f16, tag="rmv")
                rmv_ins = nc.vector.tensor_tensor(
                    out=rmV[:], in0=maskKED[:],
                    in1=bsT[:].broadcast_to([NBLK, NBLK, E + 2]),
                    op=mybir.AluOpType.mult,
                )

                # inter-block carries on PE (strided rhs views)
                nc.tensor.matmul(
                    psd[:], lhsT=ones16,
                    rhs=rmV[:, :, E:E + 1].rearrange("a k o -> a (k o)"),
                    start=False, stop=True, skip_group_check=True,
                )
                for k0, k1 in CHUNKS:
                    nc.tensor.matmul(
                        ps[:, k0:k1, :], lhsT=ones16,
                        rhs=rmV[:, k0:k1, 0:E],
                        start=False, stop=True, skip_group_check=True,
                    )

                # reciprocal on DVE
                r = rp.tile([128, NBLK], f16, tag="r")
                nc.vector.reciprocal(r[:], psd[:])
                return ps, r, rmv_ins

            def stage2(p, ps, r, next_rmv=None):
                # r4 duplication on DVE (an ACT hop here can queue
                # behind big c32/cw copies and stall the normalize)
                r4 = rp.tile([128, NBLK, 1, 4], f16, tag="r4")
                nc.vector.tensor_copy(
                    r4[:].rearrange("p k a b -> p k (a b)"),
                    r[:].to_broadcast([128, NBLK, 4]))
                # normalize straight from PSUM per carry-chunk (no ACT cw
                # hop -- the DVE->PE->ACT->DVE ping-pong left DVE idle);
                # output stores on the scalar HW ring (the sync ring
                # carries the scatters), last pair split across both rings
                ot = otp.tile([128, NBLK, E // 4, 4], f16, tag="ot")
                otv = ot[:].rearrange("p k a b -> p k (a b)")
                if p >= PAIRS - 1:
                    # the last pair gates the drain: normalize straight
                    # from PSUM per carry-chunk (no ACT cw hop)
                    for ci, (k0, k1) in enumerate(CHUNKS):
                        norm = nc.vector.tensor_tensor(
                            out=ot[:, k0:k1],
                            in0=ps[:, k0:k1, :].rearrange(
                                "p k (a b) -> p k a b", b=4),
                            in1=r4[:, k0:k1].broadcast_to(
                                [128, 8, E // 4, 4]),
                            op=mybir.AluOpType.mult,
                        )
                        if ci == 0 and next_rmv is not None:
                            add_dep_helper(norm.ins, next_rmv.ins,
                                           sync=False,
                                           reason="norm after next rm")
                        if p == PAIRS - 1:
                            if ci == 0:
                                nc.scalar.dma_start(
                                    out=out[p][:, k0:k1, :],
                                    in_=otv[:, k0:k1, :])
                            else:
                                nc.sync.dma_start(
                                    out=out[p][:, k0:k1, :],
                                    in_=otv[:, k0:k1, :])
                    if p < PAIRS - 1:
                        nc.scalar.dma_start(out=out[p], in_=otv)
                else:
                    # early pairs: ACT cw drain overlaps under later pairs
                    cw = cwp.tile([128, NBLK, E // 4, 4], f16, tag="cw")
                    nc.scalar.copy(
                        cw[:].rearrange("p k a b -> p k (a b)"), ps[:])
                    norm = nc.vector.tensor_tensor(
                        out=ot[:], in0=cw[:],
                        in1=r4[:].broadcast_to([128, NBLK, E // 4, 4]),
                        op=mybir.AluOpType.mult,
                    )
                    if next_rmv is not None:
                        add_dep_helper(norm.ins, next_rmv.ins, sync=False,
                                       reason="norm after next rm")
                    nc.scalar.dma_start(out=out[p], in_=otv)

            pending = None
            for p in allp:
                s1 = stage1(p)
                if pending is not None:
                    stage2(pending[0], pending[1][0], pending[1][1],
                           next_rmv=s1[2])
                pending = (p, s1)
            stage2(pending[0], pending[1][0], pending[1][1])

    nc.compile()
    return nc


def _get_compiled():
    global _compiled
    if _compiled is None:
        _compiled = _build()
    return _compiled


def _consts():
    f16 = np.float16
    to = np.zeros((128, 256), np.float32)
    to[:, 0:128] = np.triu(np.ones((128, 128), np.float32))  # tri[c,p]=1 iff c<=p
    to[0:16, 128:256] = 1.0  # ones16 as lhsT rows 0:16
    mk = (np.arange(NBLK)[:, None] < np.arange(NBLK)[None, :]).astype(np.float32)
    mke = np.zeros((16, NBLK, E + 2), np.float32)
    mke[:, :, 0:E + 1] = mk[:, :, None]
    mke = mke.astype(f16)
    return {
        "to_c": to.astype(f16),
        "mke_c": np.ascontiguousarray(mke),
    }


def prep_inputs(keys: np.ndarray, values: np.ndarray, w_score: np.ndarray):
    """Host-side reshard: returns in_maps (list of 8 dicts)."""
    keys = np.asarray(keys, dtype=np.float32)
    values = np.asarray(values, dtype=np.float32)
    w = np.asarray(w_score, dtype=np.float32)

    # [B,S,H,E] -> [B,H,S,E] -> [B*H, NBLK, 128, E] -> [B*H, 128, NBLK, E]
    kt = keys.transpose(0, 2, 1, 3).reshape(B * H, NBLK, 128, E)
    kt = (kt * (-SCALE * w)).transpose(0, 2, 1, 3).astype(np.float16)

    vgf = values.transpose(0, 2, 1, 3).reshape(B * H, NBLK, 128, E)
    vgf = vgf.transpose(0, 2, 1, 3).astype(np.float16)  # [B*H, 128, NBLK, E]

    consts = _consts()
    in_maps = []
    for c in range(NCORES):
        sl = slice(PAIRS * c, PAIRS * (c + 1))
        m = {
            "ktw": np.ascontiguousarray(kt[sl]),
            "vg": np.ascontiguousarray(vgf[sl]),
        }
        m.update(consts)
        in_maps.append(m)
    return in_maps


def assemble_output(results) -> np.ndarray:
    # results[c]["out"]: [PAIRS, 128, NBLK, E]; s = 128*k + partition
    arr = np.stack([np.asarray(r["out"]) for r in results])
    arr = arr.reshape(B * H, 128, NBLK, E).astype(np.float32)
    arr = arr.transpose(0, 2, 1, 3).reshape(B, H, L, E).transpose(0, 2, 1, 3)
    return np.ascontiguousarray(arr)


def kernel(queries=None, keys=None, values=None, w_score=None, b_score=None, attn_mask=None, **_):
    global LAST_RESULTS
    from concourse.bass_utils import run_bass_kernel_spmd

    nc = _get_compiled()
    in_maps = prep_inputs(keys, values, w_score)
    res = run_bass_kernel_spmd(nc, in_maps, core_ids=list(range(NCORES)), trace=TRACE)
    LAST_RESULTS = res
    return assemble_output(res.results)


# revision 5
# speedup vs baseline: 1.0736x; 1.0014x over previous
"""MinusAttention kernel for Trainium2 (8 NeuronCores, Bass/Tile) — v2.

Math: score[i,j] = (w.q_i - w.k_j + b) / sqrt(E) with causal mask.
Within a softmax row i the w.q_i and b terms cancel, so

    weights[i,j] = g_j / sum_{j'<=i} g_j',   g_j = exp(-w.k_j / sqrt(E))
    out[i,:]     = (sum_{j<=i} g_j V[j,:]) / (sum_{j<=i} g_j)

i.e. a causal cumulative weighted average of V -- O(S*E) per (b,h) --
and the output does not depend on queries at all.

Structure vs the 38.0us staged baseline (now ~35.5us median, best 34.4):
  - output stores on the HW DGE rings (scalar Q10; last pair split
    across both rings) instead of the ~80GB/s gpsimd software queue;
    the sync ring is reserved for the scatters (a 256KB store ahead of
    a scatter in the Q1 FIFO stalls the rm chain)
  - tail software-pipelined in two stages so a pair's c32 block-sum
    drain (the scatter and rm chain hang off it) never queues behind
    the previous pair's 1.1us cw copy in the ACT stream; order edges
    keep the next pair's rm ahead of the current normalize on DVE
  - the last pair normalizes straight from PSUM per carry-chunk (no
    ACT cw hop on the drain chain) and stores each half as it is ready
  - one fused mask-multiply builds the V and D carry operands at once
    (66-wide, even last dim keeps DVE packed-16 mode; strided carry
    matmul rhs views are free)
  - one exp writes the pairwise-duplicated g4 directly (stride-0 bcast
    src on ACT); the psd matmul reads a strided g4 view
  - consts packed/placed so nothing stalls: tri+ones lead the scalar
    ring, mke rides the scalar ring behind vg1; kt and vg3 on the sync
    ring (a 6th trigger on the scalar ring stalls ~1.4us on a reused
    DMA semaphore and blocks exp0 behind it in the Scalar FIFO),
    kt0 split in halves for an earlier first reduce
"""

import numpy as np

B, L, S, H, E = 4, 2048, 2048, 8, 64
NCORES = 8
PAIRS = (B * H) // NCORES  # 4 (b,h) pairs per core
NBLK = S // 128  # 16
CHUNKS = [(0, 8), (8, 16)]
SCALE = np.float32(1.0 / np.sqrt(np.float32(E)))

TRACE = False
LAST_RESULTS = None

_compiled = None


def _build():
    from concourse import bacc
    import concourse.mybir as mybir
    import concourse.tile as tile

    f16 = mybir.dt.float16
    f32 = mybir.dt.float32
    nc = bacc.Bacc("TRN2", target_bir_lowering=False, debug=False)

    ktw = nc.dram_tensor("ktw", [PAIRS, 128, NBLK, E], f16, kind="ExternalInput")
    vg = nc.dram_tensor("vg", [PAIRS, 128, NBLK, E], f16, kind="ExternalInput")
    # packed: cols 0:128 = tri (tri[c,p]=1 iff c<=p); cols 128:256 rows
    # 0:16 = ones16 (lhsT for the inter-block carry)
    to_c = nc.dram_tensor("to_c", [128, 256], f16, kind="ExternalInput")
    # maskKED[k', k, e] = 1 iff k' < k (bcast along e incl. the D col)
    mke_c = nc.dram_tensor("mke_c", [16, NBLK, E + 2], f16, kind="ExternalInput")
    out = nc.dram_tensor("out", [PAIRS, 128, NBLK, E], f16, kind="ExternalOutput")

    with tile.TileContext(nc) as tc:
        with (
            nc.allow_low_precision(reason="fp16 kernel; harness gate is 2e-2"),
            tc.tile_pool(name="const", bufs=1) as cpool,
            tc.tile_pool(name="ktp", bufs=PAIRS) as ktp,
            tc.tile_pool(name="vgp", bufs=PAIRS) as vgp,
            tc.tile_pool(name="skp", bufs=4) as skp,
            tc.tile_pool(name="gp", bufs=4) as gp,
            tc.tile_pool(name="wgp", bufs=PAIRS) as wgp,
            tc.tile_pool(name="bsp", bufs=4) as bsp,
            tc.tile_pool(name="bstp", bufs=4) as bstp,
            tc.tile_pool(name="rmp", bufs=4) as rmp,
            tc.tile_pool(name="rp", bufs=4) as rp,
            tc.tile_pool(name="cwp", bufs=4) as cwp,
            tc.tile_pool(name="otp", bufs=4) as otp,
            tc.tile_pool(name="ps", bufs=3, space="PSUM") as psp,
            tc.tile_pool(name="psd", bufs=2, space="PSUM") as psdp,
        ):
            allp = list(range(PAIRS))
            kts, vgts = {}, {}
            tos = cpool.tile([128, 256], f16)
            maskKED = cpool.tile([16, NBLK, E + 2], f16)
            tri = tos[:, 0:128]
            ones16 = tos[0:16, 128:256]
            # Sync ring: kt0 (split), kt1, consts, kt2; Scalar ring: vg0-3,
            # kt3.  Trigger instructions cost ~0.6us of engine time each,
            # so front triggers are split across both engines.
            nc.scalar.dma_start(out=tos[:], in_=to_c[:])
            for p in allp:
                kt = ktp.tile([128, NBLK, E // 4, 4], f16, tag="kt")
                vgt = vgp.tile([128, NBLK, E // 4, 4], f16, tag="vg")
                ktv = kt[:].rearrange("p k a b -> p k (a b)")
                if p == 0:
                    nc.sync.dma_start(out=ktv[:, 0:8, :], in_=ktw[p][:, 0:8, :])
                    nc.sync.dma_start(out=ktv[:, 8:16, :], in_=ktw[p][:, 8:16, :])
                else:
                    nc.sync.dma_start(out=ktv, in_=ktw[p])
                # vg3 rides the sync ring: as the 6th scalar-ring trigger
                # it stalls ~1.4us on a reused DMA semaphore and blocks
                # exp0 behind it in the Scalar FIFO (its data has slack)
                vgq = nc.sync if p == PAIRS - 1 else nc.scalar
                vgq.dma_start(
                    out=vgt[:].rearrange("p k a b -> p k (a b)"), in_=vg[p])
                kts[p], vgts[p] = kt, vgt
                if p == 1:
                    nc.scalar.dma_start(out=maskKED[:], in_=mke_c[:])

            # front phases, paced pair-by-pair on DVE/ACT; the order edge
            # keeps the scheduler from running later pairs' reduces (gated
            # on late kt DMAs) ahead of this pair's wg in the DVE stream
            from concourse.tile_rust import add_dep_helper
            wgs, g4s = {}, {}
            prev_wg = None
            for p in allp:
                t1 = skp.tile([128, NBLK, E // 8, 4], f16, tag="t1")
                sk = skp.tile([128, NBLK], f16, tag="sk")
                red = None
                for k0, k1 in (CHUNKS if p == 0 else [(0, NBLK)]):
                    red = nc.vector.tensor_tensor(
                        out=t1[:, k0:k1], in0=kts[p][:, k0:k1, 0:E // 8, :],
                        in1=kts[p][:, k0:k1, E // 8:E // 4, :],
                        op=mybir.AluOpType.add,
                    )
                    nc.vector.tensor_reduce(
                        sk[:, k0:k1], t1[:, k0:k1], mybir.AxisListType.XY,
                        mybir.AluOpType.add,
                    )
                if prev_wg is not None:
                    add_dep_helper(red.ins, prev_wg.ins, sync=False,
                                   reason="reduce after prev pair wg")
                # one exp writes the pairwise-duplicated g4 directly; its
                # (stride-1, count-4) last dim keeps the big broadcast
                # multiplies in DVE packed-16 mode
                g4 = gp.tile([128, NBLK, 1, 4], f16, tag="g4")
                nc.scalar.activation(
                    g4[:].rearrange("p k a b -> p k (a b)"),
                    sk[:].to_broadcast([128, NBLK, 4]),
                    mybir.ActivationFunctionType.Exp,
                )
                wg = wgp.tile([128, NBLK, E // 4, 4], f16, tag="wg")
                prev_wg = nc.vector.tensor_tensor(
                    out=wg[:], in0=vgts[p][:],
                    in1=g4[:].broadcast_to([128, NBLK, E // 4, 4]),
                    op=mybir.AluOpType.mult,
                )
                wgs[p] = wg
                g4s[p] = g4

            # per-pair tail, software-pipelined in two stages so the ACT
            # stream interleaves by ready-time (a pair's c32 drain must not
            # queue behind the previous pair's 1.1us cw copy -- the scatter
            # and rm chain hang off it)
            def stage1(p):
                # D prefix first: needs only g4, runs while wg is built
                psd = psdp.tile([128, NBLK], f32, tag="psd")
                nc.tensor.matmul(
                    psd[:], lhsT=tri,
                    rhs=g4s[p][:, :, 0, 0:1].rearrange("p k o -> p (k o)"),
                    start=True, stop=False, skip_group_check=True,
                )
                ps = psp.tile([128, NBLK, E], f32, tag="ps")
                for k0, k1 in CHUNKS:
                    nc.tensor.matmul(
                        ps[:, k0:k1, :], lhsT=tri,
                        rhs=wgs[p][:, k0:k1, :, :].rearrange(
                            "p k a b -> p k (a b)"),
                        start=True, stop=False, skip_group_check=True,
                    )

                # block sums live in ps row 127; PSUM reads need a
                # 32-aligned partition base: copy rows 96:128 (ACT), then
                # scatter row 31 to partitions
                c32 = bsp.tile([32, NBLK, E + 1], f16, tag="c32")
                nc.scalar.copy(c32[:, :, 0:E], ps[96:128, :, :])
                nc.scalar.copy(
                    c32[:, :, E:E + 1].rearrange("p k o -> p (k o)"),
                    psd[96:128, :])
                bsT = bstp.tile([NBLK, 1, E + 2], f16, tag="bs")
                nc.sync.dma_start(out=bsT[:, :, 0:E + 1], in_=c32[31:32, :, :],
                                  single_packet=True)

                # one fused mask-multiply covers V (cols 0:E), the D
                # col (E), and a junk col (E+1, never read) that keeps the
                # last dim even for the DVE packed-16 mode
                rmV = rmp.tile([NBLK, NBLK, E + 2], f16, tag="rmv")
                rmv_ins = nc.vector.tensor_tensor(
                    out=rmV[:], in0=maskKED[:],
                    in1=bsT[:].broadcast_to([NBLK, NBLK, E + 2]),
                    op=mybir.AluOpType.mult,
                )

                # inter-block carries on PE (strided rhs views)
                nc.tensor.matmul(
                    psd[:], lhsT=ones16,
                    rhs=rmV[:, :, E:E + 1].rearrange("a k o -> a (k o)"),
                    start=False, stop=True, skip_group_check=True,
                )
                for k0, k1 in CHUNKS:
                    nc.tensor.matmul(
                        ps[:, k0:k1, :], lhsT=ones16,
                        rhs=rmV[:, k0:k1, 0:E],
                        start=False, stop=True, skip_group_check=True,
                    )

                # reciprocal on DVE
                r = rp.tile([128, NBLK], f16, tag="r")
                nc.vector.reciprocal(r[:], psd[:])
                return ps, r, rmv_ins

            def stage2(p, ps, r, next_rmv=None):
                # r4 duplication on DVE (an ACT hop here can queue
                # behind big c32/cw copies and stall the normalize)
                r4 = rp.tile([128, NBLK, 1, 4], f16, tag="r4")
                nc.vector.tensor_copy(
                    r4[:].rearrange("p k a b -> p k (a b)"),
                    r[:].to_broadcast([128, NBLK, 4]))
                # normalize straight from PSUM per carry-chunk (no ACT cw
                # hop -- the DVE->PE->ACT->DVE ping-pong left DVE idle);
                # output stores on the scalar HW ring (the sync ring
                # carries the scatters), last pair split across both rings
                ot = otp.tile([128, NBLK, E // 4, 4], f16, tag="ot")
                otv = ot[:].rearrange("p k a b -> p k (a b)")
                if p >= PAIRS - 1:
                    # the last pair gates the drain: normalize straight
                    # from PSUM per carry-chunk (no ACT cw hop)
                    for ci, (k0, k1) in enumerate(CHUNKS):
                        norm = nc.vector.tensor_tensor(
                            out=ot[:, k0:k1],
                            in0=ps[:, k0:k1, :].rearrange(
                                "p k (a b) -> p k a b", b=4),
                            in1=r4[:, k0:k1].broadcast_to(
                                [128, 8, E // 4, 4]),
                            op=mybir.AluOpType.mult,
                        )
                        if ci == 0 and next_rmv is not None:
                            add_dep_helper(norm.ins, next_rmv.ins,
                                           sync=False,
                                           reason="norm after next rm")
                        if p == PAIRS - 1:
                            if ci == 0:
                                nc.scalar.dma_start(
                                    out=out[p][:, k0:k1, :],
                                    in_=otv[:, k0:k1, :])
                            else:
                                nc.sync.dma_start(
                                    out=out[p][:, k0:k1, :],
                                    in_=otv[:, k0:k1, :])
                    if p < PAIRS - 1:
                        nc.scalar.dma_start(out=out[p], in_=otv)
                else:
                    # early pairs: ACT cw drain overlaps under later pairs
                    cw = cwp.tile([128, NBLK, E // 4, 4], f16, tag="cw")
                    nc.scalar.copy(
                        cw[:].rearrange("p k a b -> p k (a b)"), ps[:])
                    norm = nc.vector.tensor_tensor(
                        out=ot[:], in0=cw[:],
                        in1=r4[:].broadcast_to([128, NBLK, E // 4, 4]),
                        op=mybir.AluOpType.mult,
                    )
                    if next_rmv is not None:
                        add_dep_helper(norm.ins, next_rmv.ins, sync=False,
                                       reason="norm after next rm")
                    nc.scalar.dma_start(out=out[p], in_=otv)

            pending = None
            for p in allp:
                s1 = stage1(p)
                if pending is not None:
                    stage2(pending[0], pending[1][0], pending[1][1],
                           next_rmv=s1[2])
                pending = (p, s1)
            stage2(pending[0], pending[1][0], pending[1][1])

    nc.compile()
    return nc


def _get_compiled():
    global _compiled
    if _compiled is None:
        _compiled = _build()
    return _compiled


def _consts():
    f16 = np.float16
    to = np.zeros((128, 256), np.float32)
    to[:, 0:128] = np.triu(np.ones((128, 128), np.float32))  # tri[c,p]=1 iff c<=p
    to[0:16, 128:256] = 1.0  # ones16 as lhsT rows 0:16
    mk = (np.arange(NBLK)[:, None] < np.arange(NBLK)[None, :]).astype(np.float32)
    mke = np.zeros((16, NBLK, E + 2), np.float32)
    mke[:, :, 0:E + 1] = mk[:, :, None]
    mke = mke.astype(f16)
    return {
        "to_c": to.astype(f16),
        "mke_c": np.ascontiguousarray(mke),
    }


def prep_inputs(keys: np.ndarray, values: np.ndarray, w_score: np.ndarray):
    """Host-side reshard: returns in_maps (list of 8 dicts)."""
    keys = np.asarray(keys, dtype=np.float32)
    values = np.asarray(values, dtype=np.float32)
    w = np.asarray(w_score, dtype=np.float32)

    # [B,S,H,E] -> [B,H,S,E] -> [B*H, NBLK, 128, E] -> [B*H, 128, NBLK, E]
    kt = keys.transpose(0, 2, 1, 3).reshape(B * H, NBLK, 128, E)
    kt = (kt * (-SCALE * w)).transpose(0, 2, 1, 3).astype(np.float16)

    vgf = values.transpose(0, 2, 1, 3).reshape(B * H, NBLK, 128, E)
    vgf = vgf.transpose(0, 2, 1, 3).astype(np.float16)  # [B*H, 128, NBLK, E]

    consts = _consts()
    in_maps = []
    for c in range(NCORES):
        sl = slice(PAIRS * c, PAIRS * (c + 1))
        m = {
            "ktw": np.ascontiguousarray(kt[sl]),
            "vg": np.ascontiguousarray(vgf[sl]),
        }
        m.update(consts)
        in_maps.append(m)
    return in_maps


def assemble_output(results) -> np.ndarray:
    # results[c]["out"]: [PAIRS, 128, NBLK, E]; s = 128*k + partition
    arr = np.stack([np.asarray(r["out"]) for r in results])
    arr = arr.reshape(B * H, 128, NBLK, E).astype(np.float32)
    arr = arr.transpose(0, 2, 1, 3).reshape(B, H, L, E).transpose(0, 2, 1, 3)
    return np.ascontiguousarray(arr)


def kernel(queries=None, keys=None, values=None, w_score=None, b_score=None, attn_mask=None, **_):
    global LAST_RESULTS
    from concourse.bass_utils import run_bass_kernel_spmd

    nc = _get_compiled()
    in_maps = prep_inputs(keys, values, w_score)
    res = run_bass_kernel_spmd(nc, in_maps, core_ids=list(range(NCORES)), trace=TRACE)
    LAST_RESULTS = res
    return assemble_output(res.results)
